# revision 1
# baseline (speedup 1.0000x reference)
"""Trainium2 Bass kernel for BaseMessageModule (GNN message passing).

Strategy:
- Shard ATOMS across the 8 cores (3750 each). Host routes each pair to the
  core owning its receiving atom idx_i and sorts pairs by receiving atom.
- Pairs are cut into variable-base tiles: each tile covers <= 32 consecutive
  atoms and exactly CPT*128 pair slots (pairs padded ~2%). Tile t's pairs
  accumulate into a PSUM tile addressed by t (static schedule, SPMD-safe);
  the atom base of each tile is data (host side), entering only through the
  relative atom index fed to the one-hot.
- Key linearity: reference computes (u_w @ W + b) then segment-sums; we
  segment-sum first, then apply W once per atom (20x less matmul), with the
  bias handled as count[n] * b.
- Per 128-pair chunk on device: gather embedding rows E [128p,128f] via
  dma_gather; build scaled one-hot O~ [128p, 4*32] = onehot(a_rel) *
  [w | w*u0 | w*u1 | w*u2] with two stride-0-AP vector ops; accumulate
  PSUM[f, (k,a)] += E.T @ O~ per tile.
- Tail: batched W matmul (+ b (x) count3 outer product), square/sum/sqrt for
  the vector norms, PE transposes, output DMA; host drops pad rows.

All floating-point arithmetic runs on device. Host work is integer index
manipulation (routing/sorting/padding = sharding) and array layout.
"""

import math
from contextlib import ExitStack

import numpy as np

import concourse.bass as bass
import concourse.bacc as bacc
import concourse.tile as tile
from concourse import mybir
from concourse.bass_utils import run_bass_kernel_spmd
from concourse.masks import make_identity

F = 128
ATILE = 32  # atom window per tile
KBLK = 4  # coefficient blocks: radial, u0, u1, u2
CHUNK = 128  # pairs per matmul chunk
CPT = 4  # chunks (of 128 pair slots) per tile
SUPER_T = 6  # tiles per super-iteration (gather batch = SUPER_T*CPT*128 idxs)


def _ap(t_ap, free_dims, off=0):
    """Custom AP view over the same partitions as t_ap with given free dims."""
    return bass.AP(t_ap.tensor, t_ap.offset + off, [t_ap.ap[0]] + list(free_dims))


def build_nc(N, T, n_cores):
    """Build the SPMD program for one core with T pair tiles."""
    CH = T * CPT  # chunks per core
    TOTP = CH * CHUNK  # padded pair slots per core
    UW = T * 3 * ATILE  # U region width, (t, c, a) order
    OUTR = T * ATILE  # output rows (tile-slot major, host depads)
    NBLK = OUTR // 128  # output row blocks (T kept multiple of 4)
    VW = NBLK * 128

    fp = mybir.dt.float32

    nc = bacc.Bacc("TRN2", target_bir_lowering=False, debug=False,
                   num_devices=n_cores)

    emb = nc.dram_tensor("emb", [N, F], fp, kind="ExternalInput")
    gidx = nc.dram_tensor("gidx", [128, TOTP // 16], mybir.dt.int16,
                          kind="ExternalInput")
    fT = nc.dram_tensor("fT", [128, CH], fp, kind="ExternalInput")
    r0T = nc.dram_tensor("r0T", [128, CH], fp, kind="ExternalInput")
    r1T = nc.dram_tensor("r1T", [128, CH], fp, kind="ExternalInput")
    r2T = nc.dram_tensor("r2T", [128, CH], fp, kind="ExternalInput")
    iT = nc.dram_tensor("iT", [128, CH], fp, kind="ExternalInput")
    cnt3 = nc.dram_tensor("cnt3", [1, UW], fp, kind="ExternalInput")
    wT = nc.dram_tensor("wT", [F, F], fp, kind="ExternalInput")
    brow = nc.dram_tensor("brow", [1, F], fp, kind="ExternalInput")
    out = nc.dram_tensor("out", [OUTR, 2 * F], fp, kind="ExternalOutput")

    with tile.TileContext(nc) as tc, ExitStack() as ctx:
        cpool = ctx.enter_context(tc.tile_pool(name="const", bufs=1))
        mpool = ctx.enter_context(tc.tile_pool(name="main", bufs=1))

        # --- constants ---
        iota32 = cpool.tile([128, ATILE], fp)
        nc.gpsimd.iota(iota32[:], [[1, ATILE]], base=0, channel_multiplier=0,
                       allow_small_or_imprecise_dtypes=True)
        ident = cpool.tile([128, 128], fp)
        make_identity(nc, ident[:])
        wT_sb = cpool.tile([F, F], fp)
        nc.sync.dma_start(out=wT_sb[:], in_=wT[:])
        brow_sb = cpool.tile([1, F], fp)
        nc.sync.dma_start(out=brow_sb[:], in_=brow[:])

        # --- persistent regions ---
        iT_sb = mpool.tile([128, CH], fp)
        nc.sync.dma_start(out=iT_sb[:], in_=iT[:])
        C_all = mpool.tile([128, CH * KBLK], fp)
        U = mpool.tile([128, UW], fp)  # uw segment sums, (t, c, a)
        R2 = mpool.tile([128, VW], fp)  # radial sums, slot-major
        V = mpool.tile([128, VW], fp)  # vector norms

        # --- Phase 1: batched coefficients C[p, ch, k] (scoped scratch) ---
        with tc.tile_pool(name="p1", bufs=1) as p1:
            fT_sb = p1.tile([128, CH], fp)
            nc.sync.dma_start(out=fT_sb[:], in_=fT[:])
            r0_sb = p1.tile([128, CH], fp)
            nc.sync.dma_start(out=r0_sb[:], in_=r0T[:])
            r1_sb = p1.tile([128, CH], fp)
            nc.sync.dma_start(out=r1_sb[:], in_=r1T[:])
            r2_sb = p1.tile([128, CH], fp)
            nc.sync.dma_start(out=r2_sb[:], in_=r2T[:])
            tA = p1.tile([128, CH], fp)
            tB = p1.tile([128, CH], fp)
            mult, add = mybir.AluOpType.mult, mybir.AluOpType.add
            nc.vector.tensor_tensor(out=tA[:], in0=r0_sb[:], in1=r0_sb[:], op=mult)
            nc.vector.tensor_tensor(out=tB[:], in0=r1_sb[:], in1=r1_sb[:], op=mult)
            nc.vector.tensor_tensor(out=tA[:], in0=tA[:], in1=tB[:], op=add)
            nc.vector.tensor_tensor(out=tB[:], in0=r2_sb[:], in1=r2_sb[:], op=mult)
            nc.vector.tensor_tensor(out=tA[:], in0=tA[:], in1=tB[:], op=add)
            nc.scalar.sqrt(tA[:], tA[:])  # |r|
            nc.vector.reciprocal(tB[:], tA[:])  # 1/|r|
            nc.vector.tensor_tensor(out=tB[:], in0=fT_sb[:], in1=tB[:], op=mult)
            cview = lambda k: _ap(C_all[:], [[KBLK, CH]], off=k)
            nc.vector.tensor_copy(cview(0), fT_sb[:])
            nc.vector.tensor_tensor(out=cview(1), in0=tB[:], in1=r0_sb[:], op=mult)
            nc.vector.tensor_tensor(out=cview(2), in0=tB[:], in1=r1_sb[:], op=mult)
            nc.vector.tensor_tensor(out=cview(3), in0=tB[:], in1=r2_sb[:], op=mult)

        # --- Phases 2-5 interleaved: gather/scatter supers, with the
        # W-transform (3), norms (4) and transpose/store (5) emitted as soon
        # as their input columns are flushed, so they overlap later gathers.
        n_super = math.ceil(T / SUPER_T)
        MMW = 512
        NP3 = math.ceil(UW / MMW)
        TPB = 128 // ATILE
        mult, add = mybir.AluOpType.mult, mybir.AluOpType.add
        with tc.tile_pool(name="gsup", bufs=2) as gpool, \
             tc.tile_pool(name="esup", bufs=2) as epool, \
             tc.tile_pool(name="osup", bufs=2) as opool, \
             tc.tile_pool(name="pacc", bufs=4, space="PSUM") as ppool, \
             tc.tile_pool(name="c3", bufs=2) as c3pool, \
             tc.tile_pool(name="pw", bufs=2, space="PSUM") as wpool, \
             tc.tile_pool(name="p4", bufs=2) as p4, \
             tc.tile_pool(name="ptr", bufs=2, space="PSUM") as tpool, \
             tc.tile_pool(name="ob", bufs=2) as obpool:

            gid_tiles = {}

            def load_gid(s):
                t0 = s * SUPER_T
                np_idx = min(SUPER_T, T - t0) * CPT * CHUNK
                g = gpool.tile([128, SUPER_T * CPT * CHUNK // 16],
                               mybir.dt.int16, tag="gid")
                nc.sync.dma_start(
                    out=g[:, :np_idx // 16],
                    in_=gidx[:, t0 * CPT * 8: t0 * CPT * 8 + np_idx // 16])
                gid_tiles[s] = g

            def emit_p3(k):
                c0 = k * MMW
                ncol = min(MMW, UW - c0)
                c3t = c3pool.tile([1, MMW], fp, tag="c3")
                nc.sync.dma_start(out=c3t[:1, :ncol],
                                  in_=cnt3[:1, c0:c0 + ncol])
                pw = wpool.tile([128, MMW], fp, tag="pw")
                nc.tensor.matmul(out=pw[:, :ncol], lhsT=wT_sb[:],
                                 rhs=U[:, c0:c0 + ncol], start=True,
                                 stop=False)
                nc.tensor.matmul(out=pw[:, :ncol], lhsT=brow_sb[:1, :],
                                 rhs=c3t[:1, :ncol], start=False, stop=True)
                nc.scalar.copy(U[:, c0:c0 + ncol], pw[:, :ncol])

            def emit_blk(blk):
                t0 = blk * TPB
                s0 = p4.tile([128, 128], fp, tag="s0")
                s1 = p4.tile([128, 128], fp, tag="s1")
                uvw = lambda c: _ap(U[:], [[3 * ATILE, TPB], [1, ATILE]],
                                    off=t0 * 3 * ATILE + c * ATILE)
                sv0 = _ap(s0[:], [[ATILE, TPB], [1, ATILE]])
                sv1 = _ap(s1[:], [[ATILE, TPB], [1, ATILE]])
                nc.vector.tensor_tensor(out=sv0, in0=uvw(0), in1=uvw(0), op=mult)
                nc.vector.tensor_tensor(out=sv1, in0=uvw(1), in1=uvw(1), op=mult)
                nc.vector.tensor_tensor(out=sv0, in0=s0[:], in1=s1[:], op=add)
                nc.vector.tensor_tensor(out=sv1, in0=uvw(2), in1=uvw(2), op=mult)
                nc.vector.tensor_tensor(out=sv0, in0=s0[:], in1=s1[:], op=add)
                nc.scalar.sqrt(V[:, blk * 128:(blk + 1) * 128], s0[:])
                ob = obpool.tile([128, 2 * F], fp, tag="ob")
                pt = tpool.tile([128, 128], fp, tag="pt")
                nc.tensor.matmul(out=pt[:], lhsT=V[:, blk * 128:(blk + 1) * 128],
                                 rhs=ident[:], is_transpose=True,
                                 start=True, stop=True)
                nc.scalar.copy(ob[:, 0:F], pt[:])
                pt2 = tpool.tile([128, 128], fp, tag="pt")
                nc.tensor.matmul(out=pt2[:], lhsT=R2[:, blk * 128:(blk + 1) * 128],
                                 rhs=ident[:], is_transpose=True,
                                 start=True, stop=True)
                nc.scalar.copy(ob[:, F:2 * F], pt2[:])
                nc.sync.dma_start(out=out[blk * 128:(blk + 1) * 128, :],
                                  in_=ob[:])

            load_gid(0)
            next_p3 = 0
            next_blk = 0
            for s in range(n_super):
                t0 = s * SUPER_T
                nt = min(SUPER_T, T - t0)
                sc = nt * CPT
                np_idx = sc * CHUNK
                ch0 = t0 * CPT
                if s + 1 < n_super:
                    load_gid(s + 1)
                gid_t = gid_tiles.pop(s)

                e_sup = epool.tile([128, SUPER_T * CPT * F], fp, tag="esup")
                oh_sup = opool.tile([128, SUPER_T * CPT * ATILE], fp,
                                    tag="ohsup")
                ot_sup = opool.tile([128, SUPER_T * CPT * F], fp, tag="otsup")

                nc.gpsimd.dma_gather(
                    _ap(e_sup[:], [[F, sc], [1, F]]),
                    emb[:],
                    gid_t[:, :np_idx // 16],
                    np_idx,
                    np_idx,
                    F,
                    elem_step=F,
                    single_packet=False,
                )
                nc.vector.tensor_tensor(
                    out=_ap(oh_sup[:], [[ATILE, sc], [1, ATILE]]),
                    in0=_ap(iT_sb[:], [[1, sc], [0, ATILE]], off=ch0),
                    in1=_ap(iota32[:], [[0, sc], [1, ATILE]]),
                    op=mybir.AluOpType.is_equal,
                )
                nc.vector.tensor_tensor(
                    out=_ap(ot_sup[:], [[F, sc], [ATILE, KBLK], [1, ATILE]]),
                    in0=_ap(oh_sup[:], [[ATILE, sc], [0, KBLK], [1, ATILE]]),
                    in1=_ap(C_all[:], [[KBLK, sc], [1, KBLK], [0, ATILE]],
                            off=ch0 * KBLK),
                    op=mybir.AluOpType.mult,
                )
                for ti in range(nt):
                    t = t0 + ti
                    acc = ppool.tile([128, F], fp, tag="acc")
                    for ch in range(CPT):
                        g = ti * CPT + ch
                        nc.tensor.matmul(
                            out=acc[:],
                            lhsT=_ap(e_sup[:], [[1, F]], off=g * F),
                            rhs=_ap(ot_sup[:], [[1, F]], off=g * F),
                            start=(ch == 0),
                            stop=(ch == CPT - 1),
                        )
                    nc.scalar.copy(R2[:, t * ATILE:(t + 1) * ATILE],
                                   acc[:, 0:ATILE])
                    nc.scalar.copy(U[:, t * 3 * ATILE:(t + 1) * 3 * ATILE],
                                   acc[:, ATILE:F])

                # trailing work whose inputs are now flushed
                flushed_cols = (t0 + nt) * 3 * ATILE
                while next_p3 < NP3 and (
                        (next_p3 + 1) * MMW <= flushed_cols
                        or t0 + nt == T):
                    emit_p3(next_p3)
                    next_p3 += 1
                while next_blk < NBLK and (next_blk + 1) * TPB * 3 * ATILE \
                        <= next_p3 * MMW:
                    emit_blk(next_blk)
                    next_blk += 1
            while next_blk < NBLK:
                emit_blk(next_blk)
                next_blk += 1

    nc.compile()
    return nc


def host_prep(inputs, n_cores=8):
    """Route pairs to atom-owning cores; variable-base 32-atom pair tiles."""
    emb = np.ascontiguousarray(np.asarray(inputs["atomic_embedding"],
                                          dtype=np.float32))
    f = np.asarray(inputs["f_ij_cutoff"], dtype=np.float32).ravel()
    r = np.asarray(inputs["r_ij"], dtype=np.float32)
    W = np.asarray(inputs["W"], dtype=np.float32)
    b = np.asarray(inputs["b"], dtype=np.float32)
    pl = np.asarray(inputs["pairlist"]).astype(np.int64)
    idx_i, idx_j = pl[0], pl[1]

    N = emb.shape[0]
    P = idx_i.shape[0]
    APC = N // n_cores
    SLOTS = CPT * CHUNK  # pair slots per tile

    cnt_atom = np.bincount(idx_i, minlength=N).astype(np.int64)

    # greedy variable-base tiling per core
    tiles = []  # per core: list of (astart, aend)
    for c in range(n_cores):
        ca = cnt_atom[c * APC:(c + 1) * APC]
        tl = []
        cur, cur_p = 0, 0
        for a in range(APC):
            cp = int(ca[a])
            if cur_p + cp > SLOTS or a - cur >= ATILE:
                tl.append((cur, a))
                cur, cur_p = a, 0
            cur_p += cp
        tl.append((cur, APC))
        tiles.append(tl)
    T = max(len(tl) for tl in tiles)
    T = ((T + 3) // 4) * 4  # multiple of 4 for 128-row output blocks

    # tile id and base per atom
    tile_of_atom = np.zeros(N, dtype=np.int64)
    base_of_atom = np.zeros(N, dtype=np.int64)
    for c in range(n_cores):
        for t, (a0, a1) in enumerate(tiles[c]):
            tile_of_atom[c * APC + a0:c * APC + a1] = t
            base_of_atom[c * APC + a0:c * APC + a1] = a0

    order = np.argsort(idx_i, kind="stable")
    so_i = idx_i[order]
    core_of = so_i // APC
    key = core_of * T + tile_of_atom[so_i]
    cnt = np.bincount(key, minlength=n_cores * T)
    assert cnt.max() <= SLOTS, cnt.max()
    starts = np.zeros(n_cores * T + 1, dtype=np.int64)
    np.cumsum(cnt, out=starts[1:])
    pos = np.arange(P, dtype=np.int64) - starts[key]
    slot = key * SLOTS + pos
    TOT = n_cores * T * SLOTS

    jj = np.zeros(TOT, dtype=np.int16)
    ff = np.zeros(TOT, dtype=np.float32)
    rr = np.zeros((TOT, 3), dtype=np.float32)
    rr[:, 0] = 1.0
    ii = np.zeros(TOT, dtype=np.float32)
    jj[slot] = idx_j[order]
    ff[slot] = f[order]
    rr[slot] = r[order]
    ii[slot] = (so_i - core_of * APC - base_of_atom[so_i]).astype(np.float32)

    TOTC = T * SLOTS  # padded pair slots per core
    CH = T * CPT
    in_maps = []
    out_sel = []  # per core: (valid slot rows, global atom rows)
    for c in range(n_cores):
        sl = slice(c * TOTC, (c + 1) * TOTC)
        jj_c = jj[sl]
        a16 = np.ascontiguousarray(jj_c.reshape(TOTC // 16, 16).T)
        gidx = np.ascontiguousarray(np.tile(a16, (8, 1)))
        tr = lambda x: np.ascontiguousarray(x.reshape(CH, CHUNK).T)
        cnt3 = np.zeros((T, 3, ATILE), dtype=np.float32)
        rows_slot = []
        rows_atom = []
        for t, (a0, a1) in enumerate(tiles[c]):
            span = a1 - a0
            cnt3[t, :, :span] = cnt_atom[c * APC + a0:c * APC + a1][None, :]
            rows_slot.append(np.arange(t * ATILE, t * ATILE + span))
            rows_atom.append(np.arange(c * APC + a0, c * APC + a1))
        out_sel.append((np.concatenate(rows_slot), np.concatenate(rows_atom)))
        in_maps.append({
            "emb": emb,
            "gidx": gidx,
            "fT": tr(ff[sl]),
            "r0T": tr(rr[sl][:, 0]),
            "r1T": tr(rr[sl][:, 1]),
            "r2T": tr(rr[sl][:, 2]),
            "iT": tr(ii[sl]),
            "cnt3": np.ascontiguousarray(cnt3.reshape(1, -1)),
            "wT": np.ascontiguousarray(W.T),
            "brow": np.ascontiguousarray(b.reshape(1, F)),
        })
    return in_maps, dict(N=N, APC=APC, T=T, P=P, out_sel=out_sel)


_NC_CACHE = {}


def kernel(**inputs) -> np.ndarray:
    n_cores = 8
    in_maps, meta = host_prep(inputs, n_cores)
    N = meta["N"]
    ckey = (N, meta["T"], n_cores)
    nc = _NC_CACHE.get(ckey)
    if nc is None:
        nc = build_nc(N, meta["T"], n_cores)
        _NC_CACHE[ckey] = nc
    res = run_bass_kernel_spmd(nc, in_maps, core_ids=list(range(n_cores)))
    out = np.empty((N, 2 * F), dtype=np.float32)
    for c in range(n_cores):
        rows_slot, rows_atom = meta["out_sel"][c]
        out[rows_atom] = res.results[c]["out"][rows_slot]
    return out



# revision 5
# speedup vs baseline: 1.7370x; 1.7370x over previous
"""Trainium2 Bass kernel for BaseMessageModule (GNN message passing).

Strategy:
- Shard ATOMS across the 8 cores (3750 each). Host routes each pair to the
  core owning its receiving atom idx_i and sorts pairs by receiving atom.
- Pairs are cut into variable-base tiles: each tile covers <= 32 consecutive
  atoms and exactly CPT*128 pair slots (pairs padded ~2%). Tile t's pairs
  accumulate into a PSUM tile addressed by t (static schedule, SPMD-safe);
  the atom base of each tile is data (host side), entering only through the
  relative atom index fed to the one-hot.
- Key linearity: reference computes (u_w @ W + b) then segment-sums; we
  segment-sum first, then apply W once per atom (20x less matmul), with the
  bias handled as count[n] * b.
- Prologue: the fp32 embedding table is streamed once through SBUF and
  written back to DRAM as bf16 (ebf). All PE work then runs at bf16 rate.
- Per 128-pair chunk on device: gather embedding rows E [128p,128f] (bf16)
  via dma_gather; build scaled one-hot O~ [128p, 4*32] = onehot(a_rel) *
  [w | w*u0 | w*u1 | w*u2] with two stride-0-AP vector ops; accumulate
  PSUM[f, (k,a)] += E.T @ O~ per tile.
- The gather ucode runs on one Q7 core pair per SWDGE queue; with
  num_swdge_queues=4 each super's gather is split across queues 0-3 so all
  four core pairs generate descriptors concurrently (~4x gather rate).
- Tail: batched W matmul (+ b (x) count3 outer product), square/sum/sqrt for
  the vector norms, PE transposes, output DMA; host drops pad rows.

All floating-point arithmetic runs on device. Host work is integer index
manipulation (routing/sorting/padding = sharding) and array layout.
"""

import math
from contextlib import ExitStack

import ml_dtypes
import numpy as np

import concourse.bass as bass
import concourse.bacc as bacc
import concourse.tile as tile
from concourse import mybir
from concourse.bass_utils import run_bass_kernel_spmd
from concourse.masks import make_identity

F = 128
ATILE = 32  # atom window per tile
KBLK = 4  # coefficient blocks: radial, u0, u1, u2
CHUNK = 128  # pairs per matmul chunk
CPT = 4  # chunks (of 128 pair slots) per tile
SUPER_T = 6  # tiles per super-iteration (gather batch = SUPER_T*CPT*128 idxs)
NQ = 4  # SWDGE queues (gather descriptor-gen core pairs)


def _ap(t_ap, free_dims, off=0):
    """Custom AP view over the same partitions as t_ap with given free dims."""
    return bass.AP(t_ap.tensor, t_ap.offset + off, [t_ap.ap[0]] + list(free_dims))


def build_nc(N, T, n_cores):
    """Build the SPMD program for one core with T pair tiles."""
    CH = T * CPT  # chunks per core
    TOTP = CH * CHUNK  # padded pair slots per core
    UW = T * 3 * ATILE  # U region width, (t, c, a) order
    OUTR = T * ATILE  # output rows (tile-slot major, host depads)
    NBLK = OUTR // 128  # output row blocks (T kept multiple of 4)
    VW = NBLK * 128

    fp = mybir.dt.float32
    bf = mybir.dt.bfloat16

    nc = bacc.Bacc("TRN2", target_bir_lowering=False, debug=False,
                   num_devices=n_cores, num_swdge_queues=NQ)

    emb = nc.dram_tensor("emb", [N, F], fp, kind="ExternalInput")
    ebf = nc.dram_tensor("ebf", [N, F], bf, kind="Internal")
    gidx = nc.dram_tensor("gidx", [128, TOTP // 16], mybir.dt.int16,
                          kind="ExternalInput")
    fT = nc.dram_tensor("fT", [128, CH], fp, kind="ExternalInput")
    r0T = nc.dram_tensor("r0T", [128, CH], fp, kind="ExternalInput")
    r1T = nc.dram_tensor("r1T", [128, CH], fp, kind="ExternalInput")
    r2T = nc.dram_tensor("r2T", [128, CH], fp, kind="ExternalInput")
    iT = nc.dram_tensor("iT", [128, CH], bf, kind="ExternalInput")
    cnt3 = nc.dram_tensor("cnt3", [1, UW], bf, kind="ExternalInput")
    wT = nc.dram_tensor("wT", [F, F], fp, kind="ExternalInput")
    brow = nc.dram_tensor("brow", [1, F], fp, kind="ExternalInput")
    out = nc.dram_tensor("out", [OUTR, 2 * F], fp, kind="ExternalOutput")

    NEL = N * F  # flat element count of the table
    CPP = NEL // 128  # flat elements per partition (N*F divisible by 128)
    NCONV = 4  # conversion groups
    assert CPP % NCONV == 0
    CW = CPP // NCONV

    with tile.TileContext(nc) as tc, ExitStack() as ctx:
        # --- Prologue: stream emb fp32 -> bf16 table in DRAM ---
        with tc.tile_pool(name="conv", bufs=2) as cvp:
            for g in range(NCONV):
                st = cvp.tile([128, CW], fp, tag="cin")
                nc.sync.dma_start(
                    out=st[:],
                    in_=bass.AP(emb[:].tensor, g * CW, [[CPP, 128], [1, CW]]))
                sb = cvp.tile([128, CW], bf, tag="cout")
                nc.vector.tensor_copy(sb[:], st[:])
                nc.sync.dma_start(
                    out=bass.AP(ebf[:].tensor, g * CW, [[CPP, 128], [1, CW]]),
                    in_=sb[:])

        cpool = ctx.enter_context(tc.tile_pool(name="const", bufs=1))
        mpool = ctx.enter_context(tc.tile_pool(name="main", bufs=1))

        # --- constants ---
        iota32f = cpool.tile([128, ATILE], fp)
        nc.gpsimd.iota(iota32f[:], [[1, ATILE]], base=0, channel_multiplier=0,
                       allow_small_or_imprecise_dtypes=True)
        iota32 = cpool.tile([128, ATILE], bf)
        nc.vector.tensor_copy(iota32[:], iota32f[:])
        ident = cpool.tile([128, 128], bf)
        make_identity(nc, ident[:])
        wT_sb = cpool.tile([F, F], fp)
        nc.sync.dma_start(out=wT_sb[:], in_=wT[:])
        wT_bf = cpool.tile([F, F], bf)
        nc.vector.tensor_copy(wT_bf[:], wT_sb[:])
        brow_sb = cpool.tile([1, F], fp)
        nc.sync.dma_start(out=brow_sb[:], in_=brow[:])
        brow_bf = cpool.tile([1, F], bf)
        nc.vector.tensor_copy(brow_bf[:], brow_sb[:])

        # --- persistent regions ---
        iT_sb = mpool.tile([128, CH], bf)
        nc.sync.dma_start(out=iT_sb[:], in_=iT[:])
        C_all = mpool.tile([128, CH * KBLK], bf)
        U = mpool.tile([128, UW], bf)  # uw segment sums, (t, c, a)
        R2 = mpool.tile([128, VW], bf)  # radial sums, slot-major
        V = mpool.tile([128, VW], bf)  # vector norms

        # --- Phase 1: batched coefficients C[p, ch, k] (scoped scratch) ---
        with tc.tile_pool(name="p1", bufs=1) as p1:
            fT_sb = p1.tile([128, CH], fp)
            nc.sync.dma_start(out=fT_sb[:], in_=fT[:])
            r0_sb = p1.tile([128, CH], fp)
            nc.sync.dma_start(out=r0_sb[:], in_=r0T[:])
            r1_sb = p1.tile([128, CH], fp)
            nc.sync.dma_start(out=r1_sb[:], in_=r1T[:])
            r2_sb = p1.tile([128, CH], fp)
            nc.sync.dma_start(out=r2_sb[:], in_=r2T[:])
            tA = p1.tile([128, CH], fp)
            tB = p1.tile([128, CH], fp)
            mult, add = mybir.AluOpType.mult, mybir.AluOpType.add
            nc.vector.tensor_tensor(out=tA[:], in0=r0_sb[:], in1=r0_sb[:], op=mult)
            nc.vector.tensor_tensor(out=tB[:], in0=r1_sb[:], in1=r1_sb[:], op=mult)
            nc.vector.tensor_tensor(out=tA[:], in0=tA[:], in1=tB[:], op=add)
            nc.vector.tensor_tensor(out=tB[:], in0=r2_sb[:], in1=r2_sb[:], op=mult)
            nc.vector.tensor_tensor(out=tA[:], in0=tA[:], in1=tB[:], op=add)
            nc.scalar.sqrt(tA[:], tA[:])  # |r|
            nc.vector.reciprocal(tB[:], tA[:])  # 1/|r|
            nc.vector.tensor_tensor(out=tB[:], in0=fT_sb[:], in1=tB[:], op=mult)
            cview = lambda k: _ap(C_all[:], [[KBLK, CH]], off=k)
            nc.vector.tensor_copy(cview(0), fT_sb[:])
            nc.vector.tensor_tensor(out=cview(1), in0=tB[:], in1=r0_sb[:], op=mult)
            nc.vector.tensor_tensor(out=cview(2), in0=tB[:], in1=r1_sb[:], op=mult)
            nc.vector.tensor_tensor(out=cview(3), in0=tB[:], in1=r2_sb[:], op=mult)

        # --- Phases 2-5 interleaved: gather/scatter supers, with the
        # W-transform (3), norms (4) and transpose/store (5) emitted as soon
        # as their input columns are flushed, so they overlap later gathers.
        n_super = math.ceil(T / SUPER_T)
        MMW = 512
        NP3 = math.ceil(UW / MMW)
        TPB = 128 // ATILE
        mult, add = mybir.AluOpType.mult, mybir.AluOpType.add
        with tc.tile_pool(name="gsup", bufs=2) as gpool, \
             tc.tile_pool(name="esup", bufs=2) as epool, \
             tc.tile_pool(name="osup", bufs=2) as opool, \
             tc.tile_pool(name="pacc", bufs=4, space="PSUM") as ppool, \
             tc.tile_pool(name="c3", bufs=2) as c3pool, \
             tc.tile_pool(name="pw", bufs=2, space="PSUM") as wpool, \
             tc.tile_pool(name="p4", bufs=2) as p4, \
             tc.tile_pool(name="ptr", bufs=2, space="PSUM") as tpool, \
             tc.tile_pool(name="ob", bufs=2) as obpool:

            gid_tiles = {}

            def load_gid(s):
                t0 = s * SUPER_T
                np_idx = min(SUPER_T, T - t0) * CPT * CHUNK
                g = gpool.tile([128, SUPER_T * CPT * CHUNK // 16],
                               mybir.dt.int16, tag="gid")
                nc.sync.dma_start(
                    out=g[:, :np_idx // 16],
                    in_=gidx[:, t0 * CPT * 8: t0 * CPT * 8 + np_idx // 16])
                gid_tiles[s] = g

            def emit_p3(k):
                c0 = k * MMW
                ncol = min(MMW, UW - c0)
                c3t = c3pool.tile([1, MMW], bf, tag="c3")
                nc.sync.dma_start(out=c3t[:1, :ncol],
                                  in_=cnt3[:1, c0:c0 + ncol])
                pw = wpool.tile([128, MMW], fp, tag="pw")
                nc.tensor.matmul(out=pw[:, :ncol], lhsT=wT_bf[:],
                                 rhs=U[:, c0:c0 + ncol], start=True,
                                 stop=False)
                nc.tensor.matmul(out=pw[:, :ncol], lhsT=brow_bf[:1, :],
                                 rhs=c3t[:1, :ncol], start=False, stop=True)
                nc.scalar.copy(U[:, c0:c0 + ncol], pw[:, :ncol])

            def emit_blk(blk):
                t0 = blk * TPB
                s0 = p4.tile([128, 128], fp, tag="s0")
                s1 = p4.tile([128, 128], fp, tag="s1")
                uvw = lambda c: _ap(U[:], [[3 * ATILE, TPB], [1, ATILE]],
                                    off=t0 * 3 * ATILE + c * ATILE)
                sv0 = _ap(s0[:], [[ATILE, TPB], [1, ATILE]])
                sv1 = _ap(s1[:], [[ATILE, TPB], [1, ATILE]])
                nc.vector.tensor_tensor(out=sv0, in0=uvw(0), in1=uvw(0), op=mult)
                nc.vector.tensor_tensor(out=sv1, in0=uvw(1), in1=uvw(1), op=mult)
                nc.vector.tensor_tensor(out=sv0, in0=s0[:], in1=s1[:], op=add)
                nc.vector.tensor_tensor(out=sv1, in0=uvw(2), in1=uvw(2), op=mult)
                nc.vector.tensor_tensor(out=sv0, in0=s0[:], in1=s1[:], op=add)
                nc.scalar.sqrt(V[:, blk * 128:(blk + 1) * 128], s0[:])
                ob = obpool.tile([128, 2 * F], fp, tag="ob")
                pt = tpool.tile([128, 128], bf, tag="pt")
                nc.tensor.matmul(out=pt[:], lhsT=V[:, blk * 128:(blk + 1) * 128],
                                 rhs=ident[:], is_transpose=True,
                                 start=True, stop=True)
                nc.scalar.copy(ob[:, 0:F], pt[:])
                pt2 = tpool.tile([128, 128], bf, tag="pt")
                nc.tensor.matmul(out=pt2[:], lhsT=R2[:, blk * 128:(blk + 1) * 128],
                                 rhs=ident[:], is_transpose=True,
                                 start=True, stop=True)
                nc.scalar.copy(ob[:, F:2 * F], pt2[:])
                nc.sync.dma_start(out=out[blk * 128:(blk + 1) * 128, :],
                                  in_=ob[:])

            load_gid(0)
            next_p3 = 0
            next_blk = 0
            for s in range(n_super):
                t0 = s * SUPER_T
                nt = min(SUPER_T, T - t0)
                sc = nt * CPT
                np_idx = sc * CHUNK
                ch0 = t0 * CPT
                if s + 1 < n_super:
                    load_gid(s + 1)
                gid_t = gid_tiles.pop(s)

                e_sup = epool.tile([128, SUPER_T * CPT * F], bf, tag="esup")
                oh_sup = opool.tile([128, SUPER_T * CPT * ATILE], bf,
                                    tag="ohsup")
                ot_sup = opool.tile([128, SUPER_T * CPT * F], bf, tag="otsup")

                # split the super's gather across the NQ SWDGE queues so all
                # Q7 descriptor-gen core pairs run concurrently
                for q in range(NQ):
                    c0 = q * sc // NQ
                    c1 = (q + 1) * sc // NQ
                    if c1 == c0:
                        continue
                    ni = (c1 - c0) * CHUNK
                    nc.gpsimd.dma_gather(
                        _ap(e_sup[:], [[F, c1 - c0], [1, F]], off=c0 * F),
                        ebf[:],
                        gid_t[:, c0 * 8:c1 * 8],
                        ni,
                        ni,
                        F,
                        elem_step=F,
                        single_packet=False,
                        queue_num=q,
                    )
                nc.vector.tensor_tensor(
                    out=_ap(oh_sup[:], [[ATILE, sc], [1, ATILE]]),
                    in0=_ap(iT_sb[:], [[1, sc], [0, ATILE]], off=ch0),
                    in1=_ap(iota32[:], [[0, sc], [1, ATILE]]),
                    op=mybir.AluOpType.is_equal,
                )
                nc.vector.tensor_tensor(
                    out=_ap(ot_sup[:], [[F, sc], [ATILE, KBLK], [1, ATILE]]),
                    in0=_ap(oh_sup[:], [[ATILE, sc], [0, KBLK], [1, ATILE]]),
                    in1=_ap(C_all[:], [[KBLK, sc], [1, KBLK], [0, ATILE]],
                            off=ch0 * KBLK),
                    op=mybir.AluOpType.mult,
                )
                for ti in range(nt):
                    t = t0 + ti
                    acc = ppool.tile([128, F], fp, tag="acc")
                    for ch in range(CPT):
                        g = ti * CPT + ch
                        nc.tensor.matmul(
                            out=acc[:],
                            lhsT=_ap(e_sup[:], [[1, F]], off=g * F),
                            rhs=_ap(ot_sup[:], [[1, F]], off=g * F),
                            start=(ch == 0),
                            stop=(ch == CPT - 1),
                        )
                    nc.scalar.copy(R2[:, t * ATILE:(t + 1) * ATILE],
                                   acc[:, 0:ATILE])
                    nc.scalar.copy(U[:, t * 3 * ATILE:(t + 1) * 3 * ATILE],
                                   acc[:, ATILE:F])

                # trailing work whose inputs are now flushed
                flushed_cols = (t0 + nt) * 3 * ATILE
                while next_p3 < NP3 and (
                        (next_p3 + 1) * MMW <= flushed_cols
                        or t0 + nt == T):
                    emit_p3(next_p3)
                    next_p3 += 1
                while next_blk < NBLK and (next_blk + 1) * TPB * 3 * ATILE \
                        <= next_p3 * MMW:
                    emit_blk(next_blk)
                    next_blk += 1
            while next_blk < NBLK:
                emit_blk(next_blk)
                next_blk += 1

    nc.compile()
    return nc


def host_prep(inputs, n_cores=8):
    """Route pairs to atom-owning cores; variable-base 32-atom pair tiles."""
    emb = np.ascontiguousarray(np.asarray(inputs["atomic_embedding"],
                                          dtype=np.float32))
    f = np.asarray(inputs["f_ij_cutoff"], dtype=np.float32).ravel()
    r = np.asarray(inputs["r_ij"], dtype=np.float32)
    W = np.asarray(inputs["W"], dtype=np.float32)
    b = np.asarray(inputs["b"], dtype=np.float32)
    pl = np.asarray(inputs["pairlist"]).astype(np.int64)
    idx_i, idx_j = pl[0], pl[1]

    N = emb.shape[0]
    P = idx_i.shape[0]
    APC = N // n_cores
    SLOTS = CPT * CHUNK  # pair slots per tile

    cnt_atom = np.bincount(idx_i, minlength=N).astype(np.int64)
    # cnt3 rides in bf16 (exact only for integers <= 256)
    assert cnt_atom.max() <= 256, cnt_atom.max()

    # greedy variable-base tiling per core
    tiles = []  # per core: list of (astart, aend)
    for c in range(n_cores):
        ca = cnt_atom[c * APC:(c + 1) * APC]
        tl = []
        cur, cur_p = 0, 0
        for a in range(APC):
            cp = int(ca[a])
            if cur_p + cp > SLOTS or a - cur >= ATILE:
                tl.append((cur, a))
                cur, cur_p = a, 0
            cur_p += cp
        tl.append((cur, APC))
        tiles.append(tl)
    T = max(len(tl) for tl in tiles)
    T = ((T + 3) // 4) * 4  # multiple of 4 for 128-row output blocks

    # tile id and base per atom
    tile_of_atom = np.zeros(N, dtype=np.int64)
    base_of_atom = np.zeros(N, dtype=np.int64)
    for c in range(n_cores):
        for t, (a0, a1) in enumerate(tiles[c]):
            tile_of_atom[c * APC + a0:c * APC + a1] = t
            base_of_atom[c * APC + a0:c * APC + a1] = a0

    order = np.argsort(idx_i, kind="stable")
    so_i = idx_i[order]
    core_of = so_i // APC
    key = core_of * T + tile_of_atom[so_i]
    cnt = np.bincount(key, minlength=n_cores * T)
    assert cnt.max() <= SLOTS, cnt.max()
    starts = np.zeros(n_cores * T + 1, dtype=np.int64)
    np.cumsum(cnt, out=starts[1:])
    pos = np.arange(P, dtype=np.int64) - starts[key]
    slot = key * SLOTS + pos
    TOT = n_cores * T * SLOTS

    jj = np.zeros(TOT, dtype=np.int16)
    ff = np.zeros(TOT, dtype=np.float32)
    rr = np.zeros((TOT, 3), dtype=np.float32)
    rr[:, 0] = 1.0
    ii = np.zeros(TOT, dtype=np.float32)
    jj[slot] = idx_j[order]
    ff[slot] = f[order]
    rr[slot] = r[order]
    ii[slot] = (so_i - core_of * APC - base_of_atom[so_i]).astype(np.float32)

    TOTC = T * SLOTS  # padded pair slots per core
    CH = T * CPT
    in_maps = []
    out_sel = []  # per core: (valid slot rows, global atom rows)
    for c in range(n_cores):
        sl = slice(c * TOTC, (c + 1) * TOTC)
        jj_c = jj[sl]
        a16 = np.ascontiguousarray(jj_c.reshape(TOTC // 16, 16).T)
        gidx = np.ascontiguousarray(np.tile(a16, (8, 1)))
        tr = lambda x: np.ascontiguousarray(x.reshape(CH, CHUNK).T)
        cnt3 = np.zeros((T, 3, ATILE), dtype=np.float32)
        rows_slot = []
        rows_atom = []
        for t, (a0, a1) in enumerate(tiles[c]):
            span = a1 - a0
            cnt3[t, :, :span] = cnt_atom[c * APC + a0:c * APC + a1][None, :]
            rows_slot.append(np.arange(t * ATILE, t * ATILE + span))
            rows_atom.append(np.arange(c * APC + a0, c * APC + a1))
        out_sel.append((np.concatenate(rows_slot), np.concatenate(rows_atom)))
        in_maps.append({
            "emb": emb,
            "gidx": gidx,
            "fT": tr(ff[sl]),
            "r0T": tr(rr[sl][:, 0]),
            "r1T": tr(rr[sl][:, 1]),
            "r2T": tr(rr[sl][:, 2]),
            # small exact integers: bf16 representation is lossless
            "iT": tr(ii[sl]).astype(ml_dtypes.bfloat16),
            "cnt3": np.ascontiguousarray(
                cnt3.reshape(1, -1)).astype(ml_dtypes.bfloat16),
            "wT": np.ascontiguousarray(W.T),
            "brow": np.ascontiguousarray(b.reshape(1, F)),
        })
    return in_maps, dict(N=N, APC=APC, T=T, P=P, out_sel=out_sel)


_NC_CACHE = {}


def kernel(**inputs) -> np.ndarray:
    n_cores = 8
    in_maps, meta = host_prep(inputs, n_cores)
    N = meta["N"]
    ckey = (N, meta["T"], n_cores)
    nc = _NC_CACHE.get(ckey)
    if nc is None:
        nc = build_nc(N, meta["T"], n_cores)
        _NC_CACHE[ckey] = nc
    res = run_bass_kernel_spmd(nc, in_maps, core_ids=list(range(n_cores)))
    out = np.empty((N, 2 * F), dtype=np.float32)
    for c in range(n_cores):
        rows_slot, rows_atom = meta["out_sel"][c]
        out[rows_atom] = res.results[c]["out"][rows_slot]
    return out


# revision 10
# speedup vs baseline: 1.7635x; 1.0153x over previous
"""Trainium2 Bass kernel for BaseMessageModule (GNN message passing).

Strategy:
- Shard ATOMS across the 8 cores (3750 each). Host routes each pair to the
  core owning its receiving atom idx_i and sorts pairs by receiving atom.
- Pairs are cut into variable-base tiles: each tile covers <= 32 consecutive
  atoms and exactly CPT*128 pair slots (pairs padded ~2%). Tile t's pairs
  accumulate into a PSUM tile addressed by t (static schedule, SPMD-safe);
  the atom base of each tile is data (host side), entering only through the
  relative atom index fed to the one-hot.
- Key linearity: reference computes (u_w @ W + b) then segment-sums; we
  segment-sum first, then apply W once per atom (20x less matmul), with the
  bias handled as count[n] * b.
- The fp32 embedding table is streamed once through SBUF and written back to
  DRAM as bf16 (ebf); all PE work runs at bf16 rate. While that conversion
  streams, the first BRIDGE supers gather fp32 rows directly and cast them
  per-super on the scalar engine, so descriptor generation starts at ~10us.
- Per 128-pair chunk on device: gather embedding rows E [128p,128f] (bf16)
  via dma_gather; build scaled one-hot O~ [128p, 4*32] = onehot(a_rel) *
  [w | w*u0 | w*u1 | w*u2] with vector ops; accumulate
  PSUM[f, (k,a)] += E.T @ O~ per tile.
- The gather ucode runs on one Q7 core pair per SWDGE queue; with
  num_swdge_queues=4 each super's gather is split across queues 0-3 so all
  four core pairs generate descriptors concurrently (~4x gather rate).
- Per tile the PSUM accumulator [f, 32 radial | 96 vector] is copied once
  into the combined UR region; per 4-tile block the W transform (+ b (x)
  count3) runs as one 384-col matmul, then norms + PE transposes + store.

All floating-point arithmetic runs on device. Host work is integer index
manipulation (routing/sorting/padding = sharding) and array layout.
"""

import math
from contextlib import ExitStack

import ml_dtypes
import numpy as np

import concourse.bass as bass
import concourse.bacc as bacc
import concourse.tile as tile
from concourse import mybir
from concourse.bass_utils import run_bass_kernel_spmd
from concourse.masks import make_identity

F = 128
ATILE = 32  # atom window per tile
KBLK = 4  # coefficient blocks: radial, u0, u1, u2
CHUNK = 128  # pairs per matmul chunk
CPT = 4  # chunks (of 128 pair slots) per tile
SUPER_T = 6  # tiles per super-iteration (gather batch = SUPER_T*CPT*128 idxs)
NQ = 4  # SWDGE queues (gather descriptor-gen core pairs)
BRIDGE = 12  # leading supers gathered fp32 while the bf16 table converts


def _ap(t_ap, free_dims, off=0):
    """Custom AP view over the same partitions as t_ap with given free dims."""
    return bass.AP(t_ap.tensor, t_ap.offset + off, [t_ap.ap[0]] + list(free_dims))


def build_nc(N, T, n_cores):
    """Build the SPMD program for one core with T pair tiles."""
    CH = T * CPT  # chunks per core
    TOTP = CH * CHUNK  # padded pair slots per core
    UW = T * 3 * ATILE  # cnt3 width, (t, c, a) order
    OUTR = T * ATILE  # output rows (tile-slot major, host depads)
    NBLK = OUTR // 128  # output row blocks (T kept multiple of 4)
    TPB = 128 // ATILE  # tiles per block
    VW = NBLK * 128

    fp = mybir.dt.float32
    bf = mybir.dt.bfloat16

    nc = bacc.Bacc("TRN2", target_bir_lowering=False, debug=False,
                   num_devices=n_cores, num_swdge_queues=NQ)

    emb = nc.dram_tensor("emb", [N, F], fp, kind="ExternalInput")
    ebf = nc.dram_tensor("ebf", [N, F], bf, kind="Internal")
    gidx = nc.dram_tensor("gidx", [128, TOTP // 16], mybir.dt.int16,
                          kind="ExternalInput")
    fT = nc.dram_tensor("fT", [128, CH], fp, kind="ExternalInput")
    r0T = nc.dram_tensor("r0T", [128, CH], fp, kind="ExternalInput")
    r1T = nc.dram_tensor("r1T", [128, CH], fp, kind="ExternalInput")
    r2T = nc.dram_tensor("r2T", [128, CH], fp, kind="ExternalInput")
    iT = nc.dram_tensor("iT", [128, CH], bf, kind="ExternalInput")
    cnt3 = nc.dram_tensor("cnt3", [1, UW], bf, kind="ExternalInput")
    wT = nc.dram_tensor("wT", [F, F], fp, kind="ExternalInput")
    brow = nc.dram_tensor("brow", [1, F], fp, kind="ExternalInput")
    out = nc.dram_tensor("out", [OUTR, 2 * F], fp, kind="ExternalOutput")

    NEL = N * F  # flat element count of the table
    CPP = NEL // 128  # flat elements per partition (N*F divisible by 128)
    NCONV = 4  # conversion groups
    assert CPP % NCONV == 0
    CW = CPP // NCONV

    with tile.TileContext(nc) as tc, ExitStack() as ctx:
        cpool = ctx.enter_context(tc.tile_pool(name="const", bufs=1))
        mpool = ctx.enter_context(tc.tile_pool(name="main", bufs=1))
        p1 = ctx.enter_context(tc.tile_pool(name="p1", bufs=1))

        # --- small input DMAs first so the sync queue serves them early ---
        wT_sb = cpool.tile([F, F], fp)
        nc.sync.dma_start(out=wT_sb[:], in_=wT[:])
        brow_sb = cpool.tile([1, F], fp)
        nc.sync.dma_start(out=brow_sb[:], in_=brow[:])
        iT_sb = mpool.tile([128, CH], bf)
        nc.sync.dma_start(out=iT_sb[:], in_=iT[:])
        fT_sb = p1.tile([128, CH], fp)
        nc.sync.dma_start(out=fT_sb[:], in_=fT[:])
        r0_sb = p1.tile([128, CH], fp)
        nc.sync.dma_start(out=r0_sb[:], in_=r0T[:])
        r1_sb = p1.tile([128, CH], fp)
        nc.sync.dma_start(out=r1_sb[:], in_=r1T[:])
        r2_sb = p1.tile([128, CH], fp)
        nc.sync.dma_start(out=r2_sb[:], in_=r2T[:])

        # --- prologue: stream emb fp32 -> bf16 table in DRAM (scalar queue) ---
        with tc.tile_pool(name="conv", bufs=2) as cvp:
            for g in range(NCONV):
                st = cvp.tile([128, CW], fp, tag="cin")
                nc.scalar.dma_start(
                    out=st[:],
                    in_=bass.AP(emb[:].tensor, g * CW, [[CPP, 128], [1, CW]]))
                sb = cvp.tile([128, CW], bf, tag="cout")
                nc.vector.tensor_copy(sb[:], st[:])
                nc.scalar.dma_start(
                    out=bass.AP(ebf[:].tensor, g * CW, [[CPP, 128], [1, CW]]),
                    in_=sb[:])

        # --- constants ---
        iota32f = cpool.tile([128, ATILE], fp)
        nc.gpsimd.iota(iota32f[:], [[1, ATILE]], base=0, channel_multiplier=0,
                       allow_small_or_imprecise_dtypes=True)
        iota32 = cpool.tile([128, ATILE], bf)
        nc.vector.tensor_copy(iota32[:], iota32f[:])
        ident = cpool.tile([128, 128], bf)
        make_identity(nc, ident[:])
        wT_bf = cpool.tile([F, F], bf)
        nc.vector.tensor_copy(wT_bf[:], wT_sb[:])
        brow_bf = cpool.tile([1, F], bf)
        nc.vector.tensor_copy(brow_bf[:], brow_sb[:])

        # --- persistent regions ---
        C_all = mpool.tile([128, CH * KBLK], bf)
        U = mpool.tile([128, UW], bf)  # uw segment sums, (t, c, a)
        R2 = mpool.tile([128, VW], bf)  # radial sums, slot-major
        V = mpool.tile([128, VW], bf)  # vector norms

        # --- Phase 1: batched coefficients C[p, ch, k] ---
        mult, add = mybir.AluOpType.mult, mybir.AluOpType.add
        tA = p1.tile([128, CH], fp)
        tB = p1.tile([128, CH], fp)
        nc.vector.tensor_tensor(out=tA[:], in0=r0_sb[:], in1=r0_sb[:], op=mult)
        nc.vector.tensor_tensor(out=tB[:], in0=r1_sb[:], in1=r1_sb[:], op=mult)
        nc.vector.tensor_tensor(out=tA[:], in0=tA[:], in1=tB[:], op=add)
        nc.vector.tensor_tensor(out=tB[:], in0=r2_sb[:], in1=r2_sb[:], op=mult)
        nc.vector.tensor_tensor(out=tA[:], in0=tA[:], in1=tB[:], op=add)
        nc.scalar.sqrt(tA[:], tA[:])  # |r|
        nc.vector.reciprocal(tB[:], tA[:])  # 1/|r|
        nc.vector.tensor_tensor(out=tB[:], in0=fT_sb[:], in1=tB[:], op=mult)
        cview = lambda k: _ap(C_all[:], [[KBLK, CH]], off=k)
        nc.vector.tensor_copy(cview(0), fT_sb[:])
        nc.vector.tensor_tensor(out=cview(1), in0=tB[:], in1=r0_sb[:], op=mult)
        nc.vector.tensor_tensor(out=cview(2), in0=tB[:], in1=r1_sb[:], op=mult)
        nc.vector.tensor_tensor(out=cview(3), in0=tB[:], in1=r2_sb[:], op=mult)

        # --- Phases 2-5 interleaved: gather/scatter supers, with the
        # W-transform (3), norms (4) and transpose/store (5) emitted per
        # 4-tile block as soon as its accumulators are flushed, so they
        # overlap later gathers.
        n_super = math.ceil(T / SUPER_T)
        with tc.tile_pool(name="gsup", bufs=2) as gpool, \
             tc.tile_pool(name="esup", bufs=2) as epool, \
             tc.tile_pool(name="osup", bufs=2) as opool, \
             tc.tile_pool(name="pacc", bufs=4, space="PSUM") as ppool, \
             tc.tile_pool(name="c3", bufs=2) as c3pool, \
             tc.tile_pool(name="pw", bufs=2, space="PSUM") as wpool, \
             tc.tile_pool(name="p4", bufs=2) as p4, \
             tc.tile_pool(name="ptr", bufs=2, space="PSUM") as tpool, \
             tc.tile_pool(name="ob", bufs=2) as obpool:

            gid_tiles = {}

            def load_gid(s):
                t0 = s * SUPER_T
                np_idx = min(SUPER_T, T - t0) * CPT * CHUNK
                g = gpool.tile([128, SUPER_T * CPT * CHUNK // 16],
                               mybir.dt.int16, tag="gid")
                nc.sync.dma_start(
                    out=g[:, :np_idx // 16],
                    in_=gidx[:, t0 * CPT * 8: t0 * CPT * 8 + np_idx // 16])
                gid_tiles[s] = g

            def emit_blk(blk):
                # W transform for the block's 4 tiles: one 384-col matmul
                # over the U columns + count3 (x) bias, then norms,
                # transposes and the output store.
                W3 = TPB * 3 * ATILE  # U cols per block (384)
                c3t = c3pool.tile([1, W3], bf, tag="c3")
                nc.sync.dma_start(
                    out=c3t[:], in_=cnt3[:1, blk * W3:(blk + 1) * W3])
                pw = wpool.tile([128, W3], fp, tag="pw")
                nc.tensor.matmul(out=pw[:], lhsT=wT_bf[:],
                                 rhs=U[:, blk * W3:(blk + 1) * W3],
                                 start=True, stop=False)
                nc.tensor.matmul(out=pw[:], lhsT=brow_bf[:1, :], rhs=c3t[:],
                                 start=False, stop=True)
                nc.scalar.copy(U[:, blk * W3:(blk + 1) * W3], pw[:])
                s0 = p4.tile([128, 128], fp, tag="s0")
                s1 = p4.tile([128, 128], fp, tag="s1")
                uvw = lambda c: _ap(U[:], [[3 * ATILE, TPB], [1, ATILE]],
                                    off=blk * W3 + c * ATILE)
                sv0 = _ap(s0[:], [[ATILE, TPB], [1, ATILE]])
                sv1 = _ap(s1[:], [[ATILE, TPB], [1, ATILE]])
                nc.vector.tensor_tensor(out=sv0, in0=uvw(0), in1=uvw(0), op=mult)
                nc.vector.tensor_tensor(out=sv1, in0=uvw(1), in1=uvw(1), op=mult)
                nc.vector.tensor_tensor(out=sv0, in0=s0[:], in1=s1[:], op=add)
                nc.vector.tensor_tensor(out=sv1, in0=uvw(2), in1=uvw(2), op=mult)
                nc.vector.tensor_tensor(out=sv0, in0=s0[:], in1=s1[:], op=add)
                nc.scalar.sqrt(V[:, blk * 128:(blk + 1) * 128], s0[:])
                ob = obpool.tile([128, 2 * F], fp, tag="ob")
                pt = tpool.tile([128, 128], bf, tag="pt")
                nc.tensor.matmul(out=pt[:], lhsT=V[:, blk * 128:(blk + 1) * 128],
                                 rhs=ident[:], is_transpose=True,
                                 start=True, stop=True)
                nc.scalar.copy(ob[:, 0:F], pt[:])
                pt2 = tpool.tile([128, 128], bf, tag="pt")
                nc.tensor.matmul(out=pt2[:], lhsT=R2[:, blk * 128:(blk + 1) * 128],
                                 rhs=ident[:], is_transpose=True,
                                 start=True, stop=True)
                nc.scalar.copy(ob[:, F:2 * F], pt2[:])
                nc.sync.dma_start(out=out[blk * 128:(blk + 1) * 128, :],
                                  in_=ob[:])

            load_gid(0)
            next_blk = 0
            for s in range(n_super):
                t0 = s * SUPER_T
                nt = min(SUPER_T, T - t0)
                sc = nt * CPT
                ch0 = t0 * CPT
                if s + 1 < n_super:
                    load_gid(s + 1)
                gid_t = gid_tiles.pop(s)

                e_sup = epool.tile([128, SUPER_T * CPT * F], bf, tag="esup")
                oh_sup = opool.tile([128, SUPER_T * CPT * ATILE], bf,
                                    tag="ohsup")
                ot_sup = opool.tile([128, SUPER_T * CPT * F], bf, tag="otsup")

                # split the super's gather across the NQ SWDGE queues so all
                # Q7 descriptor-gen core pairs run concurrently
                if s < BRIDGE:
                    e_fp = epool.tile([128, SUPER_T * CPT * F], fp, tag="efp")
                    for q in range(NQ):
                        c0 = q * sc // NQ
                        c1 = (q + 1) * sc // NQ
                        if c1 == c0:
                            continue
                        nc.gpsimd.dma_gather(
                            _ap(e_fp[:], [[F, c1 - c0], [1, F]], off=c0 * F),
                            emb[:],
                            gid_t[:, c0 * 8:c1 * 8],
                            (c1 - c0) * CHUNK,
                            (c1 - c0) * CHUNK,
                            F,
                            elem_step=F,
                            single_packet=False,
                            queue_num=q,
                        )
                    nc.scalar.copy(e_sup[:, :sc * F], e_fp[:, :sc * F])
                else:
                    for q in range(NQ):
                        c0 = q * sc // NQ
                        c1 = (q + 1) * sc // NQ
                        if c1 == c0:
                            continue
                        nc.gpsimd.dma_gather(
                            _ap(e_sup[:], [[F, c1 - c0], [1, F]], off=c0 * F),
                            ebf[:],
                            gid_t[:, c0 * 8:c1 * 8],
                            (c1 - c0) * CHUNK,
                            (c1 - c0) * CHUNK,
                            F,
                            elem_step=F,
                            single_packet=False,
                            queue_num=q,
                        )
                nc.vector.tensor_tensor(
                    out=_ap(oh_sup[:], [[ATILE, sc], [1, ATILE]]),
                    in0=_ap(iT_sb[:], [[1, sc], [0, ATILE]], off=ch0),
                    in1=_ap(iota32[:], [[0, sc], [1, ATILE]]),
                    op=mybir.AluOpType.is_equal,
                )
                for k in range(KBLK):
                    nc.vector.tensor_tensor(
                        out=_ap(ot_sup[:], [[F, sc], [1, ATILE]],
                                off=k * ATILE),
                        in0=_ap(oh_sup[:], [[ATILE, sc], [1, ATILE]]),
                        in1=_ap(C_all[:], [[KBLK, sc], [0, ATILE]],
                                off=ch0 * KBLK + k),
                        op=mybir.AluOpType.mult,
                    )
                for ti in range(nt):
                    t = t0 + ti
                    acc = ppool.tile([128, F], fp, tag="acc")
                    for ch in range(CPT):
                        g = ti * CPT + ch
                        nc.tensor.matmul(
                            out=acc[:],
                            lhsT=_ap(e_sup[:], [[1, F]], off=g * F),
                            rhs=_ap(ot_sup[:], [[1, F]], off=g * F),
                            start=(ch == 0),
                            stop=(ch == CPT - 1),
                        )
                    nc.vector.tensor_copy(R2[:, t * ATILE:(t + 1) * ATILE],
                                          acc[:, 0:ATILE])
                    nc.scalar.copy(U[:, t * 3 * ATILE:(t + 1) * 3 * ATILE],
                                   acc[:, ATILE:F])

                # trailing per-block work whose accumulators are flushed
                flushed_tiles = t0 + nt
                while next_blk < NBLK and (
                        (next_blk + 1) * TPB <= flushed_tiles
                        or flushed_tiles == T):
                    emit_blk(next_blk)
                    next_blk += 1
            while next_blk < NBLK:
                emit_blk(next_blk)
                next_blk += 1

    nc.compile()
    return nc


def host_prep(inputs, n_cores=8):
    """Route pairs to atom-owning cores; variable-base 32-atom pair tiles."""
    emb = np.ascontiguousarray(np.asarray(inputs["atomic_embedding"],
                                          dtype=np.float32))
    f = np.asarray(inputs["f_ij_cutoff"], dtype=np.float32).ravel()
    r = np.asarray(inputs["r_ij"], dtype=np.float32)
    W = np.asarray(inputs["W"], dtype=np.float32)
    b = np.asarray(inputs["b"], dtype=np.float32)
    pl = np.asarray(inputs["pairlist"]).astype(np.int64)
    idx_i, idx_j = pl[0], pl[1]

    N = emb.shape[0]
    P = idx_i.shape[0]
    APC = N // n_cores
    SLOTS = CPT * CHUNK  # pair slots per tile

    cnt_atom = np.bincount(idx_i, minlength=N).astype(np.int64)
    # cnt3 rides in bf16 (exact only for integers <= 256)
    assert cnt_atom.max() <= 256, cnt_atom.max()

    # greedy variable-base tiling per core
    tiles = []  # per core: list of (astart, aend)
    for c in range(n_cores):
        ca = cnt_atom[c * APC:(c + 1) * APC]
        tl = []
        cur, cur_p = 0, 0
        for a in range(APC):
            cp = int(ca[a])
            if cur_p + cp > SLOTS or a - cur >= ATILE:
                tl.append((cur, a))
                cur, cur_p = a, 0
            cur_p += cp
        tl.append((cur, APC))
        tiles.append(tl)
    T = max(len(tl) for tl in tiles)
    T = ((T + 3) // 4) * 4  # multiple of 4 for 128-row output blocks

    # tile id and base per atom
    tile_of_atom = np.zeros(N, dtype=np.int64)
    base_of_atom = np.zeros(N, dtype=np.int64)
    for c in range(n_cores):
        for t, (a0, a1) in enumerate(tiles[c]):
            tile_of_atom[c * APC + a0:c * APC + a1] = t
            base_of_atom[c * APC + a0:c * APC + a1] = a0

    order = np.argsort(idx_i, kind="stable")
    so_i = idx_i[order]
    core_of = so_i // APC
    key = core_of * T + tile_of_atom[so_i]
    cnt = np.bincount(key, minlength=n_cores * T)
    assert cnt.max() <= SLOTS, cnt.max()
    starts = np.zeros(n_cores * T + 1, dtype=np.int64)
    np.cumsum(cnt, out=starts[1:])
    pos = np.arange(P, dtype=np.int64) - starts[key]
    slot = key * SLOTS + pos
    TOT = n_cores * T * SLOTS

    jj = np.zeros(TOT, dtype=np.int16)
    ff = np.zeros(TOT, dtype=np.float32)
    rr = np.zeros((TOT, 3), dtype=np.float32)
    rr[:, 0] = 1.0
    ii = np.zeros(TOT, dtype=np.float32)
    jj[slot] = idx_j[order]
    ff[slot] = f[order]
    rr[slot] = r[order]
    ii[slot] = (so_i - core_of * APC - base_of_atom[so_i]).astype(np.float32)

    TOTC = T * SLOTS  # padded pair slots per core
    CH = T * CPT
    in_maps = []
    out_sel = []  # per core: (valid slot rows, global atom rows)
    for c in range(n_cores):
        sl = slice(c * TOTC, (c + 1) * TOTC)
        jj_c = jj[sl]
        a16 = np.ascontiguousarray(jj_c.reshape(TOTC // 16, 16).T)
        gidx = np.ascontiguousarray(np.tile(a16, (8, 1)))
        tr = lambda x: np.ascontiguousarray(x.reshape(CH, CHUNK).T)
        cnt3 = np.zeros((T, 3, ATILE), dtype=np.float32)
        rows_slot = []
        rows_atom = []
        for t, (a0, a1) in enumerate(tiles[c]):
            span = a1 - a0
            cnt3[t, :, :span] = cnt_atom[c * APC + a0:c * APC + a1][None, :]
            rows_slot.append(np.arange(t * ATILE, t * ATILE + span))
            rows_atom.append(np.arange(c * APC + a0, c * APC + a1))
        out_sel.append((np.concatenate(rows_slot), np.concatenate(rows_atom)))
        in_maps.append({
            "emb": emb,
            "gidx": gidx,
            "fT": tr(ff[sl]),
            "r0T": tr(rr[sl][:, 0]),
            "r1T": tr(rr[sl][:, 1]),
            "r2T": tr(rr[sl][:, 2]),
            # small exact integers: bf16 representation is lossless
            "iT": tr(ii[sl]).astype(ml_dtypes.bfloat16),
            "cnt3": np.ascontiguousarray(
                cnt3.reshape(1, -1)).astype(ml_dtypes.bfloat16),
            "wT": np.ascontiguousarray(W.T),
            "brow": np.ascontiguousarray(b.reshape(1, F)),
        })
    return in_maps, dict(N=N, APC=APC, T=T, P=P, out_sel=out_sel)


_NC_CACHE = {}


def kernel(**inputs) -> np.ndarray:
    n_cores = 8
    in_maps, meta = host_prep(inputs, n_cores)
    N = meta["N"]
    ckey = (N, meta["T"], n_cores)
    nc = _NC_CACHE.get(ckey)
    if nc is None:
        nc = build_nc(N, meta["T"], n_cores)
        _NC_CACHE[ckey] = nc
    res = run_bass_kernel_spmd(nc, in_maps, core_ids=list(range(n_cores)))
    out = np.empty((N, 2 * F), dtype=np.float32)
    for c in range(n_cores):
        rows_slot, rows_atom = meta["out_sel"][c]
        out[rows_atom] = res.results[c]["out"][rows_slot]
    return out
